# revision 30
# baseline (speedup 1.0000x reference)
"""Trainium2 Bass kernel for a 2-layer LSTM binary classifier.

Model: xp0 = x @ Wih0.T + b0 ; layer0 LSTM ; xp1 = seq0 @ Wih1.T + b1 ;
layer1 LSTM ; out = h1_T @ Wfc.T + bfc.

Sharding: data-parallel over batch (64 -> 8 cores x 8 examples), all
weights replicated.  Per core:
  Phase 1 (interleaved with phase 2): big input GEMM, bias added via
    ones/onehot matmuls, output xp0 stored in SBUF as [128, gate, t, b].
    Chunk 0 (t<64) runs on fp8e4 weights (its xp0 error decays through
    >190 forget gates); later chunks use float32r (full PE rate at
    N>=256, ~tf32).  Startup is latency-tuned: HWDGE generation is ONE
    shared serialized resource (~630ns/DMA) and a DMA occupies its
    queue's SEQ until HWDGE accepts it, so all DMAs sit on the sync
    queue in priority order, constants ride in one packed blob, bulk
    tensors are flat [128,X] (the DMA cost model's descriptor unit is
    the lowest AP dim), ~4us of dummy matmuls pre-ramp the PE p-state,
    and pass 0 of the GEMM accumulates straight into the window-0 PSUM
    pair (layer-0 bias via the same K=4 onehot trick as layer 1) so the
    first recurrence step issues ~10.7us after kernel start.
  Phase 2: serial recurrence, the wall-clock driver (~1.79us/step chain
    latency x 258 macro-steps).  Gates live as [gate-dim on partitions,
    batch on free].  Per macro-step u: layer0 runs step u and layer1 runs
    step u-LAG so both layers share joint elementwise instructions.
    xp contributions are pre-accumulated into PSUM per W-step window
    (identity matmul for layer0's xp0; a single K=4 gate-onehot matmul
    broadcasts layer1's bias and must be the only start=True write to the
    bank -- start=True clears the whole bank's has_written bits).  Wih1 @
    h0(v) runs as per-step matmuls that depend on LAG-old data, so they
    execute off the critical path.  tanh-gate weights are pre-scaled 2x on
    the host so sigmoid covers the g-gate too (tanh(a) = 2*sig(2a)-1,
    fixed up inside fused scalar_tensor_tensor DVE ops); the o-gate's
    sigmoid is a separate ACT op because it is only needed at the chain
    tail, so the on-chain sigmoid(i,f,g) is gated by just 6 of 8 matmuls.
    Per-step chain: 6 bf16 matmuls -> sigmoid_ifg (ACT) -> 3 DVE ops ->
    tanh (ACT) -> h-mul (DVE, bf16 out) -> next step's matmuls; ~1.79us
    of which ~1us is cross-engine sem/pipeline-drain latency.
"""

import numpy as np
import ml_dtypes

import concourse.bass as bass
import concourse.tile as tile
from concourse import bacc, mybir
from concourse.bass_utils import run_bass_kernel_spmd

F32 = mybir.dt.float32
F32R = mybir.dt.float32r
BF16 = mybir.dt.bfloat16
F8 = mybir.dt.float8e4
AF = mybir.ActivationFunctionType

H = 128          # hidden
D = 2048         # input size
B = 64           # batch
T = 256          # seq len
NCORES = 8
BS = B // NCORES          # 8 examples per core
KT = D // 128             # 16 k-tiles of the input GEMM
NCHUNK = 4                # GEMM token chunks
CTOK = T * BS // NCHUNK   # 512 tokens per chunk
TW = CTOK // BS           # 64 timesteps per chunk
W = 8                     # recurrence window (psum burst granularity)
NW = T // W
LAG = 2                   # layer1 runs LAG steps behind layer0
GORD = [0, 1, 2, 3]       # our gate order [i,f,g,o] -> torch block index
P0TOK = [64, 64, 128, 256]  # chunk-0 GEMM pass sizes (tokens)
BLOBW = 2048              # packed-constant blob width (bf16 cols)


def _build_phase2_step(nc, u, P, hwin, hinit, whh0t_s, whh1t_s, wih1t_s,
                       sig, fcT, igT, tcT, cC):
    """Emit one macro-step: layer0 step u, layer1 step u-LAG."""
    w, s = divmod(u, W)
    active = []
    if u < T:
        active.append(0)
    if u >= LAG:
        active.append(1)

    def hprev(layer, step):
        if step == 0:
            return hinit[:, layer, :]
        pu = step - 1 + (LAG if layer == 1 else 0)
        return hwin[:, (pu // W) % 2, pu % W, layer, :]

    # layer1 input projection for step v=u-LAG: depends on h0(v), which was
    # produced LAG steps ago -> executes early on PE, off the critical path
    if 1 in active:
        v = u - LAG
        h0v = hwin[:, (v // W) % 2, v % W, 0, :]
        for gi in range(4):
            nc.tensor.matmul(P[:, 1, gi, s, :], wih1t_s[:, gi, :],
                             h0v, start=False, stop=False,
                             skip_group_check=True)

    # step matmuls
    for gi in (0, 1, 2, 3):
        for l in active:
            st = u if l == 0 else u - LAG
            lhs = whh0t_s if l == 0 else whh1t_s
            nc.tensor.matmul(P[:, l, gi, s, :], lhs[:, gi, :], hprev(l, st),
                             start=False, stop=True, skip_group_check=True)

    lo = active[0]
    ln = len(active)
    L = slice(lo, lo + ln)
    # g-gate weights/bias pre-scaled by 2 on host: tanh(a) = 2*sigmoid(2a)-1,
    # so ONE sigmoid covers all 4 gates; the 2x-1 fixup fuses into the
    # scalar_tensor_tensor c-update:
    #   c = f*c + i*(2*sg-1) = fc + 2*(i*(sg-0.5))
    nc.scalar.activation(sig[:, L, 0:3, :], P[:, L, 0:3, s, :], AF.Sigmoid)
    nc.scalar.activation(sig[:, L, 3, :], P[:, L, 3, s, :], AF.Sigmoid)
    nc.vector.tensor_mul(fcT[:, L, :], sig[:, L, 1, :], cC[:, L, :])
    nc.vector.scalar_tensor_tensor(
        igT[:, L, :], sig[:, L, 2, :], 0.5, sig[:, L, 0, :],
        op0=mybir.AluOpType.subtract, op1=mybir.AluOpType.mult)
    nc.vector.scalar_tensor_tensor(
        cC[:, L, :], igT[:, L, :], 2.0, fcT[:, L, :],
        op0=mybir.AluOpType.mult, op1=mybir.AluOpType.add)
    nc.scalar.activation(tcT[:, L, :], cC[:, L, :], AF.Tanh)
    nc.vector.tensor_mul(hwin[:, w % 2, s, L, :], sig[:, L, 3, :], tcT[:, L, :])


def build_program(debug_taps=False):
    nc = bacc.Bacc("TRN2", target_bir_lowering=False, debug=False,
                   enable_asserts=False)

    # ---- DRAM I/O ----
    xd = nc.dram_tensor("xp", [NCHUNK, 128, KT, CTOK], F32R,
                        kind="ExternalInput").ap()
    # all bulk tensors are flat [128, X] so their DMA APs have a large
    # innermost dim (the cost model's descriptor unit is the lowest AP dim)
    wih0d = nc.dram_tensor("wih0t", [128, KT * 512], F32R,
                           kind="ExternalInput").ap()
    wih0f8d = nc.dram_tensor("wih0f8", [128, KT * 512], F8,
                             kind="ExternalInput").ap()
    # chunk-0 x arrives as per-pass contiguous tensors so each pass is ONE
    # cheap DMA instruction (HWDGE gen cost dominates small transfers).
    # Pass 0 (first recurrence window) is fp8 to shrink the startup DMA.
    xq0d = nc.dram_tensor("x0q0", [128, KT * P0TOK[0]], F8,
                          kind="ExternalInput").ap()
    xq_d = [None] + [nc.dram_tensor(f"x0q{i}", [128, KT * P0TOK[i]], BF16,
                                    kind="ExternalInput").ap()
                     for i in range(1, len(P0TOK))]
    # small constants packed into one blob (single DMA): cols 0:512 whh0t,
    # 512:1024 whh1t, 1024:1536 wih1t; parts 0:4: b1s@1536:1664,
    # onehot@1664:1920, b0s4@1920:2048.  The GEMM-bias pair (b0s, ones) is
    # its own tiny DMA so the first matmuls aren't behind the blob.
    blobd = nc.dram_tensor("cblob", [128, BLOBW], BF16,
                           kind="ExternalInput").ap()
    cbiasd = nc.dram_tensor("cbias", [1, 1024], BF16,
                            kind="ExternalInput").ap()
    identd = nc.dram_tensor("ident", [128, 128], F32,
                            kind="ExternalInput").ap()
    wfcd = nc.dram_tensor("wfct", [128, 1], BF16, kind="ExternalInput").ap()
    bfcd = nc.dram_tensor("bfcb", [BS, 1], F32, kind="ExternalInput").ap()
    yd = nc.dram_tensor("y", [BS, 1], F32, kind="ExternalOutput").ap()
    if debug_taps:
        dbg_xp0 = nc.dram_tensor("dbg_xp0", [128, 4, TW, BS], F32,
                                 kind="ExternalOutput").ap()
        dbg_hwin = nc.dram_tensor("dbg_hwin", [128, 2, W, 2, BS], BF16,
                                  kind="ExternalOutput").ap()
        dbg_c = nc.dram_tensor("dbg_c", [128, 2, BS], F32,
                               kind="ExternalOutput").ap()

    with tile.TileContext(nc) as tc, \
            tc.tile_pool(name="persist", bufs=1) as pp:
        # ---- persistent SBUF (bulk tensors flat, views for compute) ----
        wih0t_s = pp.tile([128, KT * 512], F32R, name="wih0t_s")
        wih0f8_s = pp.tile([128, KT * 512], F8, name="wih0f8_s")
        xq_s = [pp.tile([128, KT * n], F8 if i == 0 else BF16,
                        name=f"xq{i}_s")
                for i, n in enumerate(P0TOK)]
        blob_s = pp.tile([128, BLOBW], BF16, name="blob_s")
        cbias_s = pp.tile([1, 1024], BF16, name="cbias_s")
        ident_s = pp.tile([128, 128], F32, name="ident_s")
        wfct_s = pp.tile([128, 1], BF16, name="wfct_s")
        bfcb_s = pp.tile([BS, 1], F32, name="bfcb_s")

        def wsl(wtile, k, g):  # [K=128, 128] lhsT slice of a flat w tile
            o = (k * 4 + g) * 128
            return wtile[:, o:o + 128]

        # constant views into the blob
        whh0t_s = blob_s[:, 0:512].rearrange("p (g u) -> p g u", g=4)
        whh1t_s = blob_s[:, 512:1024].rearrange("p (g u) -> p g u", g=4)
        wih1t_s = blob_s[:, 1024:1536].rearrange("p (g u) -> p g u", g=4)
        b1s_s = blob_s[0:4, 1536:1664]
        onehot_s = blob_s[0:4, 1664:1920].rearrange("p (g n) -> p g n", g=4)
        b0s4_s = blob_s[0:4, 1920:2048]
        b0s_s = cbias_s[0:1, 0:512].rearrange("p (g u) -> p g u", g=4)
        ones_s = cbias_s[0:1, 512:1024]

        # Startup DMA schedule.  HWDGE generation is a single shared
        # serialized resource (~630ns/DMA) and a DMA instruction occupies
        # its queue's SEQ until HWDGE accepts it, so: recurrence-critical
        # DMAs first on sync in priority order, bulky weights on gpsimd
        # (SWDGE -- separate generator, keeps HWDGE free), and NOTHING on
        # the scalar/vector queues (their SEQs feed the recurrence).
        # Critical set for the first window: blob, wih0f8, xq0, ident.
        # single-queue DMA schedule in priority order (the DMA_ENGINES
        # transfer order is FIFO by generation-completion, so sync-queue
        # issue order IS the arrival order); chunk 0 uses the fp8 weights
        # throughout -- its xp0 error decays through >190 forget gates
        nc.sync.dma_start(cbias_s[:], cbiasd[:])
        nc.sync.dma_start(xq_s[0][:], xq0d[:])
        nc.sync.dma_start(wih0f8_s[:], wih0f8d[:])
        nc.sync.dma_start(blob_s[:], blobd[:])
        nc.sync.dma_start(xq_s[1][:], xq_d[1][:])
        nc.sync.dma_start(ident_s[:], identd[:])
        nc.sync.dma_start(xq_s[2][:], xq_d[2][:])
        nc.sync.dma_start(xq_s[3][:], xq_d[3][:])
        nc.sync.dma_start(wfct_s[:], wfcd[:])
        nc.sync.dma_start(bfcb_s[:], bfcd[:])
        nc.sync.dma_start(wih0t_s[:], wih0d[:])

        # xp0 per chunk: [128, gate, t-local, b] fp32
        xp0_t = [pp.tile([128, 4, TW, BS], F32, name=f"xp0_{c}")
                 for c in range(NCHUNK)]

        # recurrence state
        cC = pp.tile([128, 2, BS], F32, name="cC")
        hinit = pp.tile([128, 2, BS], BF16, name="hinit")
        hwin = pp.tile([128, 2, W, 2, BS], BF16, name="hwin")
        sig = pp.tile([128, 2, 4, BS], F32, name="sig")
        fcT = pp.tile([128, 2, BS], F32, name="fcT")
        igT = pp.tile([128, 2, BS], F32, name="igT")
        tcT = pp.tile([128, 2, BS], F32, name="tcT")
        y_sb = pp.tile([BS, 1], F32, name="y_sb")

        nc.vector.memset(cC[:], 0.0)
        nc.vector.memset(hinit[:], 0.0)
        # pre-warm the ACT function table (LoadActFuncSet ~1.3us) off-chain
        nc.scalar.activation(tcT[:, 0:1, :], cC[:, 0:1, :], AF.Sigmoid)
        nc.scalar.activation(tcT[:, 0:1, :], cC[:, 0:1, :], AF.Tanh)

        with (
            tc.tile_pool(name="xchunk", bufs=2) as x_pool,
            tc.tile_pool(name="gemm_ps", bufs=4, space="PSUM") as gemm_ps,
            tc.tile_pool(name="pair_ps", bufs=2, space="PSUM") as pair_ps,
        ):
            # ---- one chunk-0 GEMM pass: bias + k-MMs + copies ----
            def emit_pass_ops(i):
                ntok = P0TOK[i]
                toff = sum(P0TOK[:i])
                t0 = toff // BS
                tw_ = ntok // BS
                pg = []
                for g in range(4):
                    p = gemm_ps.tile([128, CTOK], F32, name="pg")
                    pg.append(p)
                    nc.tensor.matmul(p[:, 0:ntok], b0s_s[:, g, :],
                                     ones_s[:, 0:ntok],
                                     start=True, stop=False,
                                     skip_group_check=True)
                    yield 1
                for k in range(KT):
                    for g in range(4):
                        nc.tensor.matmul(
                            pg[g][:, 0:ntok], wsl(wih0f8_s, k, g),
                            xq_s[i][:, k * ntok:(k + 1) * ntok],
                            start=False, stop=(k == KT - 1),
                            skip_group_check=True)
                        yield 1
                for g in range(4):
                    dst = xp0_t[0][:, g, t0:t0 + tw_, :]
                    srcv = pg[g][:, 0:ntok].rearrange(
                        "p (t b) -> p t b", t=tw_)
                    nc.scalar.copy(dst, srcv)
                    yield 1

            # ---- GEMM op generator (pulled incrementally) ----
            def gemm_gen():
                # chunk-0 passes 1..3 (pass 0 runs in the prologue); x was
                # DMA'd up front, so only PE/copy ops are paced here
                for i in range(1, len(P0TOK)):
                    yield from emit_pass_ops(i)
                # chunks 1-3: f32r, per-k slab DMAs pace the matmuls
                for c in range(1, NCHUNK):
                    xt = x_pool.tile([128, KT, CTOK], F32R, name="xt")
                    for k in range(KT):
                        nc.sync.dma_start(xt[:, k, :], xd[c, :, k, :])
                        yield 1
                    pg = []
                    for g in range(4):
                        p = gemm_ps.tile([128, CTOK], F32, name="pg")
                        pg.append(p)
                        nc.tensor.matmul(p[:, :], b0s_s[:, g, :],
                                         ones_s[:, :],
                                         start=True, stop=False,
                                         skip_group_check=True)
                        yield 1
                    for k in range(KT):
                        for g in range(4):
                            nc.tensor.matmul(
                                pg[g][:, :], wsl(wih0t_s, k, g),
                                xt[:, k, :],
                                start=False, stop=(k == KT - 1),
                                skip_group_check=True)
                            yield 1
                    for g in range(4):
                        dst = xp0_t[c][:, g, :, :]
                        srcv = pg[g][:, :].rearrange("p (t b) -> p t b", t=TW)
                        nc.scalar.copy(dst, srcv)
                        yield 1

            gen = gemm_gen()

            def pull(n):
                for _ in range(n):
                    if next(gen, None) is None:
                        break

            # ---- prologue ----
            # PE warm-up: ~4us of dummy matmuls ramp the PE out of its cold
            # p-state so pass 0 runs at full clock the moment wih0f8 lands
            warm = gemm_ps.tile([128, CTOK], F32, name="pg")
            for _ in range(16):
                nc.tensor.matmul(warm[:, 0:256], b0s_s[:, 0, :],
                                 ones_s[:, 0:256], start=True, stop=True,
                                 skip_group_check=True)
            # pass 0 (fp8, first window) accumulates STRAIGHT INTO the
            # window-0 PSUM pair: no xp0 store, no copies, no identity
            # injection on the critical path.  Layer 0's bias lands via the
            # same K=4 onehot trick as layer 1's (ONE start=True per bank).
            P0 = pair_ps.tile([128, 2, 4, 16, BS], F32, name="pairP")
            nc.tensor.matmul(P0[:, 0, :, 0:W, :], b0s4_s[:, :],
                             onehot_s[:, :, :],
                             start=True, stop=False, skip_group_check=True)
            for k in range(KT):
                for g in range(4):
                    nc.tensor.matmul(
                        P0[:, 0, g, 0:W, :].rearrange("p s b -> p (s b)"),
                        wsl(wih0f8_s, k, g),
                        xq_s[0][:, k * W * BS:(k + 1) * W * BS],
                        start=False, stop=False, skip_group_check=True)

            P = None
            for u in range(T + LAG):
                w, s = divmod(u, W)
                if s == 0:
                    P = P0 if u == 0 else pair_ps.tile(
                        [128, 2, 4, 16, BS], F32, name="pairP")
                    if u < T and u > 0:
                        c, lw = divmod(w, TW // W)
                        nc.tensor.matmul(
                            P[:, 0, :, 0:W, :],
                            ident_s[:, :],
                            xp0_t[c][:, :, lw * W:(lw + 1) * W, :],
                            start=True, stop=False, skip_group_check=True)
                    if u + W > LAG:
                        # whole-bank bias broadcast in ONE start=True matmul
                        # (start=True clears has_written for the full bank);
                        # Wih1 @ h0 is added per-step (off the critical path).
                        nc.tensor.matmul(
                            P[:, 1, :, 0:W, :], b1s_s[:, :], onehot_s[:, :, :],
                            start=True, stop=False, skip_group_check=True)
                # delay GEMM-op interleave a few steps so the in-order PE
                # queue never stalls on a matmul whose x DMA is still in
                # flight; 8/step drains chunk-0 passes 1-3 early enough for
                # their windows while chunks 1-3 pace on their slab DMAs
                if u >= 4:
                    pull(6)
                _build_phase2_step(nc, u, P, hwin, hinit, whh0t_s, whh1t_s,
                                   wih1t_s, sig, fcT, igT, tcT, cC)
                if debug_taps and u == 31:
                    nc.sync.dma_start(dbg_xp0[:], xp0_t[0][:])
                    nc.sync.dma_start(dbg_hwin[:], hwin[:])
                    nc.sync.dma_start(dbg_c[:], cC[:])

            pull(10000)  # drain any leftovers (shouldn't be needed)

            # ---- final fc ----
            fcp = gemm_ps.tile([BS, 1], F32, name="pg")
            nc.tensor.matmul(fcp[:, :], hwin[:, (T + LAG - 1) // W % 2,
                                             (T + LAG - 1) % W, 1, :],
                             wfct_s[:, :], start=True, stop=True,
                             skip_group_check=True)
            nc.scalar.activation(y_sb[:, :], fcp[:, :], AF.Identity,
                                 bias=bfcb_s[:, :])
            nc.sync.dma_start(yd[:], y_sb[:])

    nc.compile()
    return nc


_PROG = None


def _get_program():
    global _PROG
    if _PROG is None:
        _PROG = build_program()
    return _PROG


def prep_inputs(x, Wih0, Whh0, bih0, bhh0, Wih1, Whh1, bih1, bhh1, Wfc, bfc):
    """Host-side layout prep -> per-core in_maps."""
    bf = ml_dtypes.bfloat16
    f8 = ml_dtypes.float8_e4m3
    x = np.asarray(x, np.float32)

    # weights: [4H, K] -> [K(part), gate(ours), unit]
    def gate_T(Wmat):  # [512, K] -> [K, 4, 128] in our gate order
        A = np.asarray(Wmat, np.float32).reshape(4, 128, -1)  # tg, j, k
        A = A.transpose(2, 0, 1)[:, GORD, :]                  # k, ours, j
        A = A.copy()
        A[:, 2, :] *= 2.0  # tanh-gate folded 2x (tanh(a)=2*sig(2a)-1)
        return np.ascontiguousarray(A)

    wih0t = gate_T(Wih0).reshape(KT, 128, 4, 128).transpose(1, 0, 2, 3)
    wih0t = np.ascontiguousarray(wih0t, np.float32)           # [128,KT,4,128]
    whh0t = gate_T(Whh0).astype(bf)                           # [128,4,128]
    whh1t = gate_T(Whh1).astype(bf)
    wih1t = gate_T(Wih1).astype(bf)

    b0 = (np.asarray(bih0) + np.asarray(bhh0)).astype(np.float32)
    b1 = (np.asarray(bih1) + np.asarray(bhh1)).astype(np.float32)
    b0s = b0.reshape(4, 128)[GORD].copy()
    b0s[2] *= 2.0                                             # [4,128]
    b1g4 = b1.reshape(4, 128)[GORD].copy()
    b1g4[2] *= 2.0                                            # [4,128]
    ident = np.eye(128, dtype=np.float32)
    wfct = np.asarray(Wfc, np.float32).T.astype(bf)           # [128,1]
    bfcb = np.full((BS, 1), np.asarray(bfc, np.float32)[0], np.float32)

    # packed constant blob (single startup DMA); layout must match the
    # blob_s views in build_program
    blob = np.zeros((128, BLOBW), np.float32)
    blob[:, 0:512] = whh0t.reshape(128, 512)
    blob[:, 512:1024] = whh1t.reshape(128, 512)
    blob[:, 1024:1536] = wih1t.reshape(128, 512)
    blob[0:4, 1536:1664] = b1g4
    blob[0:4, 1664:1920] = np.repeat(np.eye(4, dtype=np.float32),
                                     W * BS).reshape(4, 4 * W * BS)
    blob[0:4, 1920:2048] = b0s
    blob = blob.astype(bf)
    cbias = np.concatenate([b0s.reshape(512),
                            np.ones(512, np.float32)])[None].astype(bf)

    wih0flat = wih0t.reshape(128, KT * 512)
    common = dict(wih0t=wih0flat,
                  wih0f8=wih0flat.astype(f8),
                  cblob=blob, cbias=cbias, ident=ident, wfct=wfct,
                  bfcb=bfcb)

    offs = np.cumsum([0] + P0TOK)
    in_maps = []
    for c in range(NCORES):
        xs = x[c * BS:(c + 1) * BS]                           # [BS, T, D]
        xt = xs.transpose(2, 1, 0).reshape(D, T * BS)         # [d, tok(t,b)]
        xpre = (xt.reshape(KT, 128, NCHUNK, CTOK)
                .transpose(2, 1, 0, 3))                       # [c,128,k,tok]
        m = {"xp": np.ascontiguousarray(xpre, np.float32), **common}
        for i in range(len(P0TOK)):
            seg = np.ascontiguousarray(xpre[0][:, :, offs[i]:offs[i + 1]])
            m[f"x0q{i}"] = (seg.astype(f8 if i == 0 else bf)
                            .reshape(128, KT * P0TOK[i]))
        in_maps.append(m)
    return in_maps


def run(inputs, **kw):
    nc = _get_program()
    in_maps = prep_inputs(**inputs)
    res = run_bass_kernel_spmd(nc, in_maps, core_ids=list(range(NCORES)), **kw)
    y = np.concatenate([res.results[c]["y"] for c in range(NCORES)], axis=0)
    return y.astype(np.float32), res


def kernel(**inputs):
    y, _ = run(inputs)
    return y


if __name__ == "__main__":
    import sys
    if "--sim" in sys.argv:
        import trails.perfetto as _tp
        if not hasattr(_tp.LazyPerfetto, "add_counter"):
            def _add_counter(self, proc, track, ts_, val):
                self.update_counter(proc, track, int(ts_), float(val),
                                    unit="ns")
            _tp.LazyPerfetto.add_counter = _add_counter
        for _m in ("enable_explicit_ordering", "reserve_process_order"):
            if not hasattr(_tp.LazyPerfetto, _m):
                setattr(_tp.LazyPerfetto, _m,
                        lambda self, *a, **k: None)
        from concourse.timeline_sim import TimelineSim
        nc = _get_program()
        ts = TimelineSim(nc, trace="--trace" in sys.argv)
        dur = ts.simulate()
        print(f"TimelineSim predicted duration: {dur:.0f} ns")
        if ts.perfetto is not None:
            ts.perfetto.save("/root/problem/timeline.pftrace")
            print("wrote /root/problem/timeline.pftrace")

